# revision 29
# baseline (speedup 1.0000x reference)
"""Llama3 attention layer (T=2048, 32 q heads / 8 kv heads, D=128, hidden 4096)
on 8 Trainium2 NeuronCores, tensor-parallel over heads.

Per-core shard: 4 q heads + 1 kv head (w_qkv columns), 512 w_o rows.
Each core computes a full [T, 4096] o_proj partial in bf16; the host sums
the 8 partials in f32 (the all-reduce of the row-parallel w_o).

Device algorithm (per core), matmuls in bf16 with fp32 PSUM accumulation:
  1. qkv^T = w_shard^T @ hs^T          -> [768, T]  (c on partitions)
     cb order: k first, then q0..q3, then v (k's rope off critical path)
  2. RoPE on q^T/k^T rows via duplicated cos/sin tables (DVE)
  3. V = transpose(v^T) via PE transposes
  4. S^T[s,t] = k^T.T @ q^T per head; exp on ACT into a wide P tile;
     0/1 mask on diagonal blocks (DVE)
  5. out^T[d,t] += V[s].T @ P^T[s,t]; denominator via DVE accumulation of
     P blocks + one ones-matmul per (head, tile); reciprocal_approx_fast;
     normalize reads PSUM directly
  6. o_proj emitted as 32 "units" per tile, interleaved into the NEXT
     tile's QKV/attention phases to keep the PE saturated
"""
import math
from collections import deque

import numpy as np
import ml_dtypes

import bass_rust
import concourse.bass as bass
import concourse.mybir as mybir
import concourse.tile as tile
from concourse.bass_utils import run_bass_kernel_spmd
from concourse.masks import make_identity
from concourse.vector_clock import ScopedClock

BF16 = mybir.dt.bfloat16
F32 = mybir.dt.float32
bf16 = ml_dtypes.bfloat16

T = 2048
HID = 4096
D = 128
NQH = 4           # q heads per core
CB = 6            # qkv col blocks of 128 (k, 4 q heads, v)
HCH = HID // 128  # 32 hidden chunks
TJ = 512          # t tile width
NJ = T // TJ      # 4 t tiles
SB = T // 128     # 16 s blocks
SCALE = 1.0 / math.sqrt(D)

_MAX_CTRL_WAITS = 1


def _install_drain_fix():
    """walrus in this image allows only 1 sem wait on CTRL (nop/drain)
    instructions; spread the Tile tail-drain's global-clock waits across
    preceding sync-engine NOPs."""
    if getattr(tile.TileContext, "_drain_fix_installed", False):
        return

    def _patched(self, tick_clock, wait_clock):
        nc = self.nc
        nops = [nc.sync.nop(nofuse=True, hint=f"drainw{i}") for i in range(32)]
        drain_inst = nc.sync.drain()
        wait_clock.add_sem_waits(
            drain_inst.ins, ScopedClock({None: tick_clock.global_clock})
        )
        si = drain_inst.ins.sync_info
        waits = list(si.on_wait) if si and si.on_wait else []
        if len(waits) > _MAX_CTRL_WAITS:
            chunks = [
                waits[i:i + _MAX_CTRL_WAITS]
                for i in range(0, len(waits), _MAX_CTRL_WAITS)
            ]
            drain_inst.ins.sync_info = bass_rust.SyncInfo(
                on_wait=chunks[-1], on_update=list(si.on_update or [])
            )
            for nop, chunk in zip(nops, chunks[:-1]):
                nop.ins.sync_info = bass_rust.SyncInfo(on_wait=chunk, on_update=[])
        nc.all_engine_barrier()
        assert self.sems is not None
        popped = nc._tile_sem_poison_stack.pop()
        assert popped is self._sem_poison
        nc.clear_and_free_semaphores(list(self.sems.allocated().values()))
        nc.all_engine_barrier()

    tile.TileContext._drain_and_barrier = _patched
    tile.TileContext._drain_fix_installed = True


def _fix_bir_waits(bir_json: bytes, max_waits: int = 1) -> bytes:
    """walrus in this image accepts very few sem waits per instruction.
    Split any instruction carrying more than `max_waits` waits by inserting
    same-engine NoOps ahead of it that carry the excess waits."""
    import json

    bir = json.loads(bir_json)
    for fn in bir["functions"]:
        for blk in fn["blocks"]:
            out = []
            for inst in blk["instructions"]:
                si = inst.get("sync_info")
                waits = (si or {}).get("on_wait") or []
                if len(waits) > max_waits:
                    chunks = [
                        waits[i:i + max_waits]
                        for i in range(0, len(waits), max_waits)
                    ]
                    for k, ch in enumerate(chunks[:-1]):
                        out.append(
                            {
                                "debug": inst.get("debug", 0),
                                "engine": inst["engine"],
                                "ins": [],
                                "name": f"{inst['name']}-w{k}",
                                "opcode": "NoOp",
                                "outs": [],
                                "sync_info": {"on_update": [], "on_wait": ch},
                            }
                        )
                    si["on_wait"] = chunks[-1]
                out.append(inst)
            blk["instructions"] = out
    return json.dumps(bir).encode()


def build_nc() -> bass.Bass:
    _install_drain_fix()
    nc = bass.Bass()

    hsT_d = nc.dram_tensor("hsT", [128, HCH, T], BF16, kind="ExternalInput")
    w_d = nc.dram_tensor("wqkv", [128, CB, HCH, 128], BF16, kind="ExternalInput")
    wo_d = nc.dram_tensor("wo", [128, NQH, HID], BF16, kind="ExternalInput")
    cos_d = nc.dram_tensor("cos2", [128, T], BF16, kind="ExternalInput")
    sin_d = nc.dram_tensor("sin2", [128, T], BF16, kind="ExternalInput")
    mask_d = nc.dram_tensor("masks", [128, 4, TJ], BF16, kind="ExternalInput")
    out_d = nc.dram_tensor("out", [T, HID], BF16, kind="ExternalOutput")

    with tile.TileContext(nc) as tc:
        with (
            tc.tile_pool(name="const", bufs=1) as constp,
            tc.tile_pool(name="acts", bufs=1) as actp,
            tc.tile_pool(name="hst", bufs=1) as hstp,
            tc.tile_pool(name="qp", bufs=8) as qp,
            tc.tile_pool(name="otp", bufs=8) as otp,
            tc.tile_pool(name="Pp", bufs=2) as Pp,
            tc.tile_pool(name="qkt", bufs=2) as qktp,
            tc.tile_pool(name="rtmp", bufs=3) as rtp,
            tc.tile_pool(name="accp", bufs=4) as accp,
            tc.tile_pool(name="rcp", bufs=4) as rcpp,
            tc.tile_pool(name="outp", bufs=2) as outp,
            tc.tile_pool(name="pss", bufs=3, space="PSUM") as pssp,
            tc.tile_pool(name="pso", bufs=2, space="PSUM") as psop,
            tc.tile_pool(name="shp", bufs=3, space="PSUM") as shp,
        ):
            cos_sb = constp.tile([128, T], BF16, tag="cos")
            sin_sb = constp.tile([128, T], BF16, tag="sin")
            mask_sb = constp.tile([128, 4, TJ], BF16, tag="mask")
            ones_sb = constp.tile([128, 128], BF16, tag="ones")
            ident_sb = constp.tile([128, 128], BF16, tag="ident")

            # persistent activations
            w_sb = actp.tile([128, CB, HCH, 128], BF16, tag="w")
            wo_sb = actp.tile([128, NQH, HID], BF16, tag="wo")
            kT_sb = actp.tile([128, T], BF16, tag="kT")
            vT_sb = actp.tile([128, T], BF16, tag="vT")
            v_sb = [
                actp.tile([128, 128], BF16, tag=f"v{i}", name=f"v{i}")
                for i in range(SB)
            ]

            # ---- initial DMA schedule (j=0): interleave w / hst / consts so
            # the first matmuls start as early as possible.
            hst0 = hstp.tile([128, HCH, TJ], BF16, tag="hst", name="hst0")
            for q in range(4):
                o8 = slice(8 * q, 8 * q + 8)
                nc.sync.dma_start(w_sb[:, 0, o8, :], w_d[:, 0, o8, :])
                nc.sync.dma_start(hst0[:, o8, :], hsT_d[:, o8, 0:TJ])
            nc.sync.dma_start(w_sb[:, 1, :, :], w_d[:, 1, :, :])
            nc.sync.dma_start(cos_sb[:], cos_d[:])
            nc.sync.dma_start(sin_sb[:], sin_d[:])
            nc.sync.dma_start(w_sb[:, 2, :, :], w_d[:, 2, :, :])
            nc.sync.dma_start(w_sb[:, 3, :, :], w_d[:, 3, :, :])
            nc.sync.dma_start(mask_sb[:], mask_d[:])
            nc.sync.dma_start(w_sb[:, 4, :, :], w_d[:, 4, :, :])
            nc.sync.dma_start(w_sb[:, 5, :, :], w_d[:, 5, :, :])
            nc.vector.memset(ones_sb[:], 1.0)
            make_identity(nc, ident_sb[:])

            # ---- o_proj unit machinery -------------------------------------
            # A unit computes out[t128, n*512:(n+1)*512] for one t block of
            # tile j from ot tiles + wo, staging into a [128, HID] bf16 tile
            # DMA'd per half-row. Units for tile j are emitted interleaved
            # into tile j+1's QKV/attention phases (j=NJ-1's at the end).
            pending = deque()
            ob_tiles = {}

            def emit_unit():
                if not pending:
                    return False
                jj, tl, n, ot_tiles = pending.popleft()
                key = (jj, tl)
                if key not in ob_tiles:
                    ob_tiles[key] = outp.tile(
                        [128, HID], BF16, tag="ob", name=f"ob{jj}_{tl}"
                    )
                ob = ob_tiles[key]
                ps = shp.tile([128, TJ], F32, tag="ps", name="ps_u")
                tloc = bass.ts(tl, 128)
                for c in range(NQH):
                    nc.tensor.matmul(
                        ps[:], ot_tiles[c][:, tloc], wo_sb[:, c, bass.ts(n, TJ)],
                        start=(c == 0), stop=(c == NQH - 1),
                    )
                if n % 2 == 0:
                    nc.scalar.copy(ob[:, bass.ts(n, TJ)], ps[:])
                else:
                    nc.vector.tensor_copy(ob[:, bass.ts(n, TJ)], ps[:])
                if n == 3 or n == HID // TJ - 1:
                    tg = 4 * jj + tl
                    half = bass.ts(n // 4, HID // 2)
                    nc.sync.dma_start(out_d[bass.ts(tg, 128), half], ob[:, half])
                    if n == HID // TJ - 1:
                        del ob_tiles[key]
                return True

            # cb emission order: k first (rope for k completes while q
            # matmuls run), then q0..q3, then v.  Host packs w in this order.
            hst_tiles = [hst0, None, None, None]
            q_tiles_all = {}
            pending_finish = [None]

            def get_q_tiles(j):
                if j not in q_tiles_all:
                    q_tiles_all[j] = [
                        qp.tile([128, TJ], BF16, tag="q", name=f"q{j}_{h}")
                        for h in range(NQH)
                    ]
                return q_tiles_all[j]

            def run_finish():
                if pending_finish[0] is not None:
                    fin = pending_finish[0]
                    pending_finish[0] = None
                    fin()

            def rope(ps, cb, j):
                """psum -> qkt copy -> rotary -> kT (cb==0) or q tile."""
                js = bass.ts(j, TJ)
                qk_t = qktp.tile([128, TJ], BF16, tag="qkt")
                nc.scalar.copy(qk_t[:], ps[:])
                # partition-half swap on the (idle) Pool engine frees the DVE
                # for attention-phase work; rope chains have plenty of slack
                swp = rtp.tile([128, TJ], BF16, tag="swp")
                nc.gpsimd.tensor_copy(swp[0:64, :], qk_t[64:128, :])
                nc.gpsimd.tensor_copy(swp[64:128, :], qk_t[0:64, :])
                ta = rtp.tile([128, TJ], BF16, tag="ta")
                nc.vector.tensor_tensor(
                    ta[:], qk_t[:], cos_sb[:, js], mybir.AluOpType.mult
                )
                tb = rtp.tile([128, TJ], BF16, tag="tb")
                nc.vector.tensor_tensor(
                    tb[:], swp[:], sin_sb[:, js], mybir.AluOpType.mult
                )
                dst = kT_sb[:, js] if cb == 0 else get_q_tiles(j)[cb - 1][:]
                nc.vector.tensor_tensor(dst, ta[:], tb[:], mybir.AluOpType.add)

            def emit_qkv_cb(j, cb, finish_at=None):
                ps = shp.tile([128, TJ], F32, tag="ps", name="ps_qkv")
                for h in range(HCH):
                    nc.tensor.matmul(
                        ps[:], w_sb[:, cb, h, :], hst_tiles[j][:, h, :],
                        start=(h == 0), stop=(h == HCH - 1),
                    )
                    if h == finish_at:
                        run_finish()
                if cb < 5:
                    rope(ps, cb, j)
                else:
                    nc.vector.tensor_copy(vT_sb[:, bass.ts(j, TJ)], ps[:])

            def make_qkv_granules(j, cb):
                """Split one QKV column block into 4-matmul fill granules."""
                state = {}

                def mk(chunk):
                    def g():
                        if chunk == 0:
                            state["ps"] = shp.tile(
                                [128, TJ], F32, tag="ps", name="ps_qkv"
                            )
                        ps = state["ps"]
                        for h in range(4 * chunk, 4 * chunk + 4):
                            nc.tensor.matmul(
                                ps[:], w_sb[:, cb, h, :], hst_tiles[j][:, h, :],
                                start=(h == 0), stop=(h == HCH - 1),
                            )
                        if chunk == 7:
                            rope(state["ps"], cb, j)
                    return g

                return [mk(c) for c in range(8)]

            fill_q = deque()

            def fill(k):
                for _ in range(k):
                    if fill_q:
                        fill_q.popleft()()
                    elif not emit_unit():
                        return

            for j in range(NJ):
                js = bass.ts(j, TJ)
                nblk = 4 * j + 4
                ngrp = nblk // 2

                q_tiles = get_q_tiles(j)
                ot_tiles = [
                    otp.tile([128, TJ], BF16, tag="ot", name=f"ot{j}_{h}")
                    for h in range(NQH)
                ]

                # ---- QKV^T for this t tile (cb 0/1 of j=1 were pulled into
                # attention(0) as fill granules) ----
                first_cb = 2 if j == 1 else 0
                for cb in range(first_cb, CB):
                    emit_qkv_cb(j, cb, finish_at=15 if cb == first_cb else None)

                # ---- V blocks for this tile ----
                for i in range(4 * j, 4 * j + 4):
                    pv = shp.tile([128, 128], BF16, tag="ps", name="ps_vt")
                    nc.tensor.transpose(pv[:], vT_sb[:, bass.ts(i, 128)], ident_sb[:])
                    nc.vector.tensor_copy(v_sb[i][:], pv[:])

                # prefetch hst for j+1; wo during j=0
                if j + 1 < NJ:
                    nxt = hstp.tile(
                        [128, HCH, TJ], BF16, tag="hst", name=f"hst{j + 1}"
                    )
                    njs = bass.ts(j + 1, TJ)
                    nc.sync.dma_start(nxt[:, 0:16, :], hsT_d[:, 0:16, njs])
                    nc.sync.dma_start(nxt[:, 16:32, :], hsT_d[:, 16:32, njs])
                    hst_tiles[j + 1] = nxt
                if j == 0:
                    nc.sync.dma_start(wo_sb[:], wo_d[:])
                    # att(0) has no o_proj units yet: fill it with the first
                    # two column blocks of QKV(1) instead
                    fill_q.extend(make_qkv_granules(1, 0))
                    fill_q.extend(make_qkv_granules(1, 1))

                # ---- attention: S-pairs one group ahead of PV; grouped exp
                # over [128, 1024] (2 PSUM banks) halves ACT per-tile cost;
                # fill (o_proj units / QKV granules) keeps the PE busy while
                # the ACT engine works through the exps ----
                slots_left = 4 * ngrp
                for h in range(NQH):
                    qT = q_tiles[h]
                    P = Pp.tile([128, 8, TJ], BF16, tag="P", name=f"P{j}_{h}")
                    ps_o = psop.tile([128, TJ], F32, tag="ps", name="ps_o")
                    acc = [None, None]

                    def t0_of(i):
                        # causal: diagonal block i (s in [128i, 128i+128))
                        # only contributes to t >= 128(i-4j) within the tile
                        return 128 * (i - 4 * j) if i >= 4 * j else 0

                    def do_s(i):
                        t0 = t0_of(i)
                        ps_s = pssp.tile([128, TJ], F32, tag="ps", name="ps_s")
                        nc.tensor.matmul(
                            ps_s[:, t0:], kT_sb[:, bass.ts(i, 128)], qT[:, t0:],
                            start=True, stop=True,
                        )
                        nc.scalar.activation(
                            P[:, i % 8, t0:], ps_s[:, t0:],
                            mybir.ActivationFunctionType.Exp, scale=SCALE,
                        )
                        if i >= 4 * j:
                            nc.vector.tensor_tensor(
                                P[:, i % 8, t0:], P[:, i % 8, t0:],
                                mask_sb[:, i - 4 * j, t0:],
                                mybir.AluOpType.mult,
                            )

                    def do_pv(i):
                        t0 = t0_of(i)
                        nc.tensor.matmul(
                            ps_o[:, t0:], v_sb[i][:], P[:, i % 8, t0:],
                            start=(i == 0), stop=(i == nblk - 1),
                            skip_group_check=True,
                        )
                        a = i % 2
                        if acc[a] is None:
                            acc[a] = accp.tile(
                                [128, TJ], BF16, tag="acc", name=f"acc{a}"
                            )
                            if t0 == 0:
                                nc.vector.tensor_copy(acc[a][:], P[:, i % 8, :])
                            else:
                                nc.vector.memset(acc[a][:, 0:t0], 0.0)
                                nc.vector.tensor_copy(
                                    acc[a][:, t0:], P[:, i % 8, t0:]
                                )
                        else:
                            nc.vector.tensor_tensor(
                                acc[a][:, t0:], acc[a][:, t0:], P[:, i % 8, t0:],
                                mybir.AluOpType.add,
                            )

                    def do_slot():
                        items = len(fill_q) + len(pending)
                        k = -(-items // max(slots_left, 1))
                        fill(min(k, 3))

                    for i in range(nblk):
                        do_s(i)
                        if i == (6 if nblk >= 12 else 4):
                            run_finish()
                        if i >= 2:
                            if i % 2 == 0:
                                do_slot()
                                slots_left -= 1
                            do_pv(i - 2)
                    do_slot()
                    slots_left -= 1
                    if nblk == 4:
                        run_finish()
                    do_pv(nblk - 2)
                    do_pv(nblk - 1)

                    def make_finish(h, acc, ps_o, ot_h):
                        def fin():
                            nc.vector.tensor_tensor(
                                acc[0][:], acc[0][:], acc[1][:],
                                mybir.AluOpType.add,
                            )
                            ps_den = shp.tile(
                                [128, TJ], F32, tag="ps", name="ps_den"
                            )
                            nc.tensor.matmul(
                                ps_den[:], ones_sb[:], acc[0][:],
                                start=True, stop=True,
                            )
                            # rc = 1/den via exp(-ln(den)) on ACT (ln and exp
                            # share a table; DVE InstReciprocal is 3.3us and
                            # custom-DVE ops don't compile on this walrus)
                            ld = rcpp.tile([128, TJ], F32, tag="rc", name="ld")
                            nc.scalar.activation(
                                ld[:], ps_den[:],
                                mybir.ActivationFunctionType.Ln,
                            )
                            rc = rcpp.tile([128, TJ], F32, tag="rc", name="rc")
                            nc.scalar.activation(
                                rc[:], ld[:],
                                mybir.ActivationFunctionType.Exp, scale=-1.0,
                            )
                            nc.vector.tensor_tensor(
                                ot_h[:], ps_o[:], rc[:], mybir.AluOpType.mult
                            )
                        return fin

                    pending_finish[0] = make_finish(h, acc, ps_o, ot_tiles[h])

                # any unconsumed QKV granules must be emitted before the next
                # tile's S matmuls read the kT/q they produce
                while fill_q:
                    fill_q.popleft()()

                # queue this tile's o_proj units (consumed as fill in the
                # next tile's attention phase)
                for tl in range(4):
                    for n in range(HID // TJ):
                        pending.append((j, tl, n, ot_tiles))

            run_finish()
            while emit_unit():
                pass

    _orig_to_json = nc.to_json_bytes

    def _patched_to_json():
        return _fix_bir_waits(_orig_to_json())

    nc.to_json_bytes = _patched_to_json
    return nc


_NC_CACHE = None


def _get_nc():
    global _NC_CACHE
    if _NC_CACHE is None:
        _NC_CACHE = build_nc()
    return _NC_CACHE


def _host_prep(positions, hidden_states, w_qkv, w_o):
    H, HKV = 32, 8
    pos = np.asarray(positions).astype(np.float32)
    inv_freq = 1.0 / (500000.0 ** (np.arange(0, D, 2, dtype=np.float32) / D))
    freqs = pos[:, None] * inv_freq[None, :]                  # [T, 64]
    cos = np.cos(freqs).T                                     # [64, T]
    sin = np.sin(freqs).T
    cos2 = np.ascontiguousarray(
        np.concatenate([cos, cos], 0)
    ).astype(bf16)                                            # [128, T]
    sin2 = np.ascontiguousarray(np.concatenate([-sin, sin], 0)).astype(bf16)

    # diagonal 0/1 masks: [p, r, f] = ((128r + p) <= f)
    p = np.arange(128)[:, None, None]
    r = np.arange(4)[None, :, None]
    f = np.arange(TJ)[None, None, :]
    masks = np.ascontiguousarray(
        ((128 * r + p) <= f).astype(np.float32)
    ).astype(bf16)                                            # [128, 4, 512]

    hs = np.asarray(hidden_states)
    # [p, o, t]
    hsT = np.ascontiguousarray(
        hs.T.reshape(HCH, 128, T).transpose(1, 0, 2)
    ).astype(bf16)
    w_qkv = np.asarray(w_qkv)
    w_o = np.asarray(w_o)

    in_maps = []
    for core in range(8):
        qc = slice(core * 4 * D, (core + 1) * 4 * D)
        kc = slice(H * D + core * D, H * D + (core + 1) * D)
        vc = slice((H + HKV) * D + core * D, (H + HKV) * D + (core + 1) * D)
        # cb order: k, q0..q3, v
        wshard = np.concatenate(
            [w_qkv[:, kc], w_qkv[:, qc], w_qkv[:, vc]], axis=1
        )                                                     # [4096, 768]
        # [p, cb, o, c]
        wshard = np.ascontiguousarray(
            wshard.reshape(HCH, 128, CB, 128).transpose(1, 2, 0, 3)
        ).astype(bf16)
        # [p, c, n]
        woshard = np.ascontiguousarray(
            w_o[core * 512:(core + 1) * 512, :]
            .reshape(NQH, 128, HID)
            .transpose(1, 0, 2)
        ).astype(bf16)
        in_maps.append(
            {
                "hsT": hsT,
                "wqkv": wshard,
                "wo": woshard,
                "cos2": cos2,
                "sin2": sin2,
                "masks": masks,
            }
        )
    return in_maps


def kernel(positions, hidden_states, w_qkv, w_o, _trace=False):
    nc = _get_nc()
    in_maps = _host_prep(positions, hidden_states, w_qkv, w_o)
    res = run_bass_kernel_spmd(nc, in_maps, list(range(8)), trace=_trace)
    out = np.zeros((T, HID), np.float32)
    for c in range(8):
        out += res.results[c]["out"].astype(np.float32)
    if _trace:
        kernel._last_result = res
    return out
